# revision 43
# baseline (speedup 1.0000x reference)
"""Trainium2 Bass kernel for nn_MultiHeadAttention_7413113553038.

Sharding: 8 cores = (batch b in {0,1}) x (query block of 512). Each core
computes all 4 heads of attention for its 512 queries against the full 2048
keys of its batch, plus the output projection, residual add and LayerNorm for
its rows. No collectives needed.

Per-core strategy:
  - Q/K/V activations and the QKV projection weights ship as fp8e4m3 (values
    are O(1); the quantization noise averages out over the 2048-key softmax
    sum), halving the startup-critical DMA bytes. The 1/sqrt(d_k) score
    scale is applied inside the EXP stage instead of being folded into W_Q
    (whose values would underflow fp8). Residual, fc weights, gauss tables
    and output stay f16.
  - The QKV projections run as single DoubleRow fp8 matmuls (both 128-row
    chunks of the F=256 contraction per instruction; the [Ki, chunk, *]
    host packing is already the DoubleRow operand layout).
  - Input DMAs spread over three queues in need-order: scalar carries ONLY
    xqt so the ACT exp stream starts as soon as the first scores land;
    the K-side streams on sync, the V-side + epilogue tensors on gpsimd.
  - Q^T/K^T computed in [d, seq] layout; scores computed transposed:
    sT[k, q] = K Q^T per head (contraction d=64, head pairs at partition
    bases 0/64 issue as concurrent PE row-tiles).
  - Softmax is max-free (scores are O(8)) with a uniform e^-3 shift that
    cancels in p/Z. Per (group, k-chunk) the exp runs as one of:
      * ACT f16 chunks (band slots kc<=5): exp on the scalar engine, then
        the multiplicative Gaussian band table E = exp(g) applied to both
        heads in one DVE op (tables are 1 beyond each head's band).
      * ACT fp8 chunks (beyond-band): exp straight to an fp8e4m3 pair
        tile (max p = e^(smax-3) ~ 270 < 448, no E multiply); each pair
        feeds ONE DoubleRow PV matmul per head (2 k-chunks per pass).
      * DVE chunks (late in each group, when the projection casts are
        done): Schraudolph exp - one tensor_scalar builds f16 bits as a
        uint16 affine of the f32 scores (negative affine clamps to +0.0,
        i.e. p underflows to exactly 0); softmax cancels the constant-
        offset miscalibration. Relieves the ACT engine, which is the
        cadence anchor.
  - V is stored fp8 (fp8 lhsT works for both the f16-rhs and DoubleRow PV
    matmuls) and augmented with a ones-column so ctxT = V_aug.T @ p
    accumulates the softmax denominator Z as psum row 64 for free; the
    V tile inner dim is padded to 68 so the k-chunk stride is 16B-aligned
    for DoubleRow weight loads.
  - Z handling: Z rows copied to partition 0, one fused custom-DVE
    reciprocal straight to f16, then a ones-column matmul per head
    broadcasts 1/Z to the head's partitions (tile_position picks the
    base) and one DVE multiply scales ctxT in place.
  - fc: ctxT [dm, q] is the lhsT the fc matmul needs; the residual (+
    group-0 fc partials) is accumulated into the fc psum with an
    identity-matmul. LayerNorm runs per fc-pair (stats on DVE, rstd via
    ACT sqrt + DVE reciprocal, normalize split ACT/DVE) so the first
    half's normalize/DMA overlaps the second half's stats; the sqrt
    table load hides in the ACT-idle window after the last exp.
  - Both attention loops are software-pipelined by one chunk (scores for
    kc+1 are emitted before PV of kc); group 1 carries the K projections
    it needs plus group 0's dripped epilogue in its PE slack, and its
    first five score/exp chunks are emitted ahead of group 0's psum
    drain. Output is f16 (upcast on host), two DMAs per LayerNorm half.
"""

import numpy as np

N_HEADS = 4
D_K = 64
B = 2
S = 2048
F = 256
QB = 512  # queries per core
P = 128
KC = S // P  # 16 k-chunks
SIGMA_HS = (5.0, 10.0, 20.0, 40.0)
LN_EPS = 1e-5
N_CORES = 8
# per-head causal-bias band width (g >= ~1e-4): ceil(4.292 * sigma)
BAND = (22, 43, 86, 172)
E01_W = 192
E25_W = 304


_CACHE = {}


def _gauss_tables():
    """Multiplicative Gaussian-bias band tables E = exp(g) in fp16,
    transposed-score layout (delta = q - k = off_t + j - i, off_t = 256-128t).

      e01 [4,128,192]: e01[h,i,m] = exp(g_h(m - i + 128)), slots 0,1
                       (slice col = (128 - 128t) + j)
      e25 [4,128,304]: e25[h,i,m] = exp(g_h(m - i)), slots 2..5
                       (slice col = j - 128*(t-2))
    g_h(d) = exp(-d^2 / (2 sigma_h^2)) for d >= 0 else 0.
    """
    i = np.arange(P, dtype=np.float64)[None, :, None]
    sig = np.asarray(SIGMA_HS, dtype=np.float64)[:, None, None]

    m01 = np.arange(E01_W, dtype=np.float64)[None, None, :]
    d01 = m01 - i + 128.0
    g01 = np.where(d01 >= 0, np.exp(-(d01 ** 2) / (2 * sig ** 2)), 0.0)

    m25 = np.arange(E25_W, dtype=np.float64)[None, None, :]
    d25 = m25 - i
    g25 = np.where(d25 >= 0, np.exp(-(d25 ** 2) / (2 * sig ** 2)), 0.0)
    return (
        np.exp(g01).astype(np.float16),
        np.exp(g25).astype(np.float16),
    )


def _build_program():
    import concourse.bass as bass  # noqa: F401
    import concourse.tile as tile
    from concourse import bacc, mybir
    from concourse.dve_ops import (
        RECIP_APPROX_FAST_CONSTS,
        RECIPROCAL_APPROX_FAST,
    )
    from concourse.masks import make_identity

    f32 = mybir.dt.float32
    f16 = mybir.dt.float16
    u16 = mybir.dt.uint16
    i32 = mybir.dt.int32
    f8 = mybir.dt.float8e4
    AF = mybir.ActivationFunctionType
    ALU = mybir.AluOpType
    DR = mybir.MatmulPerfMode.DoubleRow

    # Schraudolph exp->f16-bits constants (softmax cancels any constant
    # offset in B, so rounding-mode miscalibration is harmless). The e^-3
    # shift keeps the fp8 chunks under the e4m3 max; it cancels in p/Z.
    SHIFT = 3.0
    EXP_A = (1024.0 / np.log(2.0)) * 0.125  # fold the 1/sqrt(d_k) scale
    EXP_B = 1024.0 * (15.0 - 0.043677) - SHIFT * (1024.0 / np.log(2.0))

    nc = bacc.Bacc("TRN2", target_bir_lowering=False, debug=False)

    # pre-packed inputs; K/V side in fp8
    xkt = nc.dram_tensor("xkt", [P, 4, 2, 512], f8, kind="ExternalInput").ap()
    xqt = nc.dram_tensor("xqt", [P, 2, QB], f8, kind="ExternalInput").ap()
    g01 = nc.dram_tensor("g01", [P, N_HEADS, E01_W], f16, kind="ExternalInput").ap()
    g25 = nc.dram_tensor("g25", [P, N_HEADS, E25_W], f16, kind="ExternalInput").ap()
    res = nc.dram_tensor("res", [P, 4, F], f16, kind="ExternalInput").ap()
    w8 = nc.dram_tensor("w8", [P, 3, 2, F], f8, kind="ExternalInput").ap()
    wf = nc.dram_tensor("wf", [P, 2, F], f16, kind="ExternalInput").ap()
    xvt = nc.dram_tensor("xvt", [P, 4, 2, 512], f8, kind="ExternalInput").ap()
    out = nc.dram_tensor("out", [P, 4, F], f16, kind="ExternalOutput").ap()

    with tile.TileContext(nc) as tc:
        with (
            tc.tile_pool(name="xin", bufs=1) as xin,
            tc.tile_pool(name="proj", bufs=1) as proj,
            tc.tile_pool(name="mmps", bufs=2, space="PSUM") as mmps,
            tc.tile_pool(name="spsum", bufs=2, space="PSUM") as spsum,
            tc.tile_pool(name="cpsum", bufs=2, space="PSUM") as cpsum,
            tc.tile_pool(name="ptpool", bufs=6) as ptpool,
        ):
            # ---- input loads: scalar queue carries ONLY xqt (so the ACT
            # exp stream starts as soon as the first scores land); the
            # K-side streams on sync, the V-side + epilogue tensors on
            # gpsimd, all in need-order ----
            xqt_sb = xin.tile([P, 2, QB], f8, tag="xqt")
            nc.scalar.dma_start(xqt_sb, xqt)
            w8_sb = xin.tile([P, 3, 2, F], f8, tag="w8")
            nc.sync.dma_start(w8_sb[:, 0:2], w8[:, 0:2])
            nc.gpsimd.dma_start(w8_sb[:, 2:3], w8[:, 2:3])
            xk0_sb = xin.tile([P, 2, 512], f8, tag="xk0")
            nc.sync.dma_start(xk0_sb, xkt[:, 0])
            xv0_sb = xin.tile([P, 2, 512], f8, tag="xv0")
            nc.gpsimd.dma_start(xv0_sb, xvt[:, 0])
            g01_sb = xin.tile([P, N_HEADS, E01_W], f16, tag="g01")
            nc.sync.dma_start(g01_sb, g01)
            xv123_sb = xin.tile([P, 3, 2, 512], f8, tag="xv123")
            nc.gpsimd.dma_start(xv123_sb[:, 0], xvt[:, 1])
            g25_sb = xin.tile([P, N_HEADS, E25_W], f16, tag="g25")
            nc.sync.dma_start(g25_sb, g25)
            xk123_sb = xin.tile([P, 3, 2, 512], f8, tag="xk123")
            nc.sync.dma_start(xk123_sb[:, 0], xkt[:, 1])
            nc.gpsimd.dma_start(xv123_sb[:, 1], xvt[:, 2])
            nc.sync.dma_start(xk123_sb[:, 1], xkt[:, 2])
            nc.gpsimd.dma_start(xv123_sb[:, 2], xvt[:, 3])
            nc.sync.dma_start(xk123_sb[:, 2], xkt[:, 3])
            res_sb = xin.tile([P, 4, F], f16, tag="res")
            nc.gpsimd.dma_start(res_sb, res)
            wf_sb = xin.tile([P, 2, F], f16, tag="wf")
            nc.gpsimd.dma_start(wf_sb, wf)

            # ---- persistent tiles ----
            qt_sb = proj.tile([P, 2, QB], f16, tag="qt")
            kt_sb = proj.tile([P, 4, 2, 512], f16, tag="kt")
            v_sb = proj.tile([P, KC, N_HEADS, 68], f8, tag="v")  # 68: 16B-aligned kc stride for DoubleRow
            ctx_sb = proj.tile([P, 2, QB], f16, tag="ctx")
            fcacc = proj.tile([P, 4, F], f16, tag="fcacc")
            o_sb = proj.tile([P, 4, F], f16, tag="osb")
            ztmp32 = proj.tile([1, N_HEADS, QB], f32, tag="ztmp32")
            rz16 = proj.tile([1, N_HEADS, QB], f16, tag="rz16")
            ones16 = proj.tile([1, 64], f16, tag="ones16")
            ident = proj.tile([P, P], f16, tag="ident")
            eps_t = proj.tile([P, 1], f32, tag="eps")
            st_t = proj.tile([P, 4, 6], f32, tag="st")
            mv_t = proj.tile([P, 4, 2], f32, tag="mv")
            lnt = proj.tile([P, 4], f32, tag="lnt")
            rstd = proj.tile([P, 4], f32, tag="rstd")
            nbias = proj.tile([P, 4], f32, tag="nbias")
            negone = proj.tile([P, 1], f32, tag="negone")
            nshift = proj.tile([P, 1], f32, tag="nshift")
            veps = proj.tile([P, 4], f32, tag="veps")
            rt1 = proj.tile([P, 4], f32, tag="rt1")

            nc.vector.memset(ones16, 1.0)
            nc.vector.memset(eps_t, LN_EPS)
            nc.vector.memset(negone, -1.0)
            nc.vector.memset(nshift, -SHIFT)
            nc.vector.memset(v_sb[:, :, :, 64:65], 1.0)
            make_identity(nc, ident)  # gpsimd-only


            # ---- projection helpers (DoubleRow fp8: both 128-chunks of the
            # F=256 contraction in one matmul; w8/x layouts are already
            # [Ki, ktile, *]) ----
            def proj_k(nb, g, split_cast=False):
                ps = mmps.tile([P, 512], f32, tag="mm", name=f"psk{nb}{g}")
                xk = xk0_sb if nb == 0 else xk123_sb[:, nb - 1]
                nc.tensor.matmul(
                    ps,
                    w8_sb[:, 0, :, g * P:(g + 1) * P],
                    xk,
                    start=True,
                    stop=True,
                    perf_mode=DR,
                )
                if split_cast:
                    nc.vector.tensor_copy(kt_sb[:, nb, g, 0:P], ps[:, 0:P])
                    nc.vector.tensor_copy(kt_sb[:, nb, g, P:], ps[:, P:])
                else:
                    nc.vector.tensor_copy(kt_sb[:, nb, g, :], ps)

            def proj_q(g, on_scalar=False):
                ps = mmps.tile([P, 512], f32, tag="mm", name=f"psq{g}")
                nc.tensor.matmul(
                    ps,
                    w8_sb[:, 1, :, g * P:(g + 1) * P],
                    xqt_sb,
                    start=True,
                    stop=True,
                    perf_mode=DR,
                )
                if on_scalar:
                    nc.scalar.copy(qt_sb[:, g, :], ps)
                else:
                    nc.vector.tensor_copy(qt_sb[:, g, :], ps)

            def proj_v(jp):
                """V projection for the q-chunk pair (2jp, 2jp+1): two
                DoubleRow matmuls into one [P, 512] psum, one copy out."""
                ps = mmps.tile([P, 512], f32, tag="mm", name=f"psv{jp}")
                for i in range(2):
                    j = 2 * jp + i
                    nb, jj = divmod(j, 4)
                    xv = xv0_sb if nb == 0 else xv123_sb[:, nb - 1]
                    nc.tensor.matmul(
                        ps[:, i * F:(i + 1) * F],
                        xv[:, :, jj * P:(jj + 1) * P],
                        w8_sb[:, 2],
                        start=True,
                        stop=True,
                        perf_mode=DR,
                    )
                for i in range(2):
                    nc.vector.tensor_copy(
                        v_sb[:, 2 * jp + i, :, 0:64],
                        ps[:, i * F:(i + 1) * F].rearrange(
                            "p (h d) -> p h d", h=N_HEADS
                        ),
                    )

            # ---- attention ----
            # Chunk kinds per (group, kc):
            #   'f16': ACT exp -> f16 pt (band slots kc<=5, E multiply after)
            #   'dve': DVE Schraudolph -> uint16 f16-bits (neg affine clamps
            #          to +0.0, i.e. p underflows to exactly 0)
            #   'fp8': ACT exp -> fp8 pair tile; PV runs as one DoubleRow
            #          matmul per pair (beyond-band only: no E multiply, so
            #          max p = e^(smax-3) ~ 270 < fp8 max 448)
            # All chunks share the e^-3 shift, which cancels in p/Z.
            DVE_KC = {0: (13, 15), 2: (9, 11)}
            PAIRS = {0: ((6, 7), (8, 9), (10, 11), (12, 14)),
                     2: ((6, 7), (8, 10), (12, 13), (14, 15))}
            SEQ = tuple(range(KC))
            PAIR_OF = {
                g: {kc: (pr, j) for pr in prs for j, kc in enumerate(pr)}
                for g, prs in PAIRS.items()
            }

            def kind_of(G, kc):
                if kc in DVE_KC[G[0]]:
                    return "dve"
                return "f16" if kc <= 5 else "fp8"

            pts = {}

            def attn_scores(G, kc):
                ps = spsum.tile([P, 2 * QB], f32, tag="sc", name=f"sc{G[0]}_{kc}")
                for hi, h in enumerate(G):
                    g, po = h // 2, (h % 2) * 64
                    nc.tensor.matmul(
                        ps[:, hi * QB:(hi + 1) * QB],
                        kt_sb[po:po + 64, kc // 4, g, (kc % 4) * P:(kc % 4 + 1) * P],
                        qt_sb[po:po + 64, g, :],
                        start=True,
                        stop=True,
                    )
                return ps

            def emit_sc(G, kc):
                ps = attn_scores(G, kc)
                kind = kind_of(G, kc)
                if kind == "dve":
                    pt = ptpool.tile([P, 2, QB], f16, tag="pt",
                                     name=f"pt{G[0]}_{kc}")
                    nc.vector.tensor_scalar(
                        pt.rearrange("p a b -> p (a b)").bitcast(u16),
                        ps, EXP_A, EXP_B,
                        op0=ALU.mult, op1=ALU.add,
                    )
                    attn_band(G, kc, pt)
                    pts[(G[0], kc)] = pt
                elif kind == "f16":
                    pt = ptpool.tile([P, 2, QB], f16, tag="pt",
                                     name=f"pt{G[0]}_{kc}")
                    nc.scalar.activation(
                        pt.rearrange("p a b -> p (a b)"), ps, AF.Exp,
                        bias=nshift, scale=0.125,
                    )
                    attn_band(G, kc, pt)
                    pts[(G[0], kc)] = pt
                else:
                    pr, j = PAIR_OF[G[0]][kc]
                    if j == 0:
                        pts[(G[0], pr)] = ptpool.tile(
                            [P, 2, 2, QB], f8, tag="pt",
                            name=f"p8_{G[0]}_{pr[0]}",
                        )
                    t8 = pts[(G[0], pr)]
                    nc.scalar.activation(
                        t8[:, j].rearrange("p a b -> p (a b)"), ps, AF.Exp,
                        bias=nshift, scale=0.125,
                    )

            def attn_band(G, kc, pt):
                """Multiplicative Gaussian band, both heads in one op
                (tables are exp(g)=1 beyond each head's band). G1's wide
                slots run on the otherwise-idle gpsimd engine."""
                if kc > 5:
                    return
                off_t = 256 - 128 * kc
                j0 = max(0, -off_t)
                j1 = min(512, max(BAND[h] for h in G) + 128 - off_t)
                j1 = min(512, (j1 + 7) & ~7)
                if j1 <= j0:
                    return
                if kc <= 1:
                    c0 = (128 - 128 * kc) + j0
                    esl = g01_sb[:, G[0]:G[0] + 2, c0:c0 + (j1 - j0)]
                else:
                    c0 = j0 - 128 * (kc - 2)
                    esl = g25_sb[:, G[0]:G[0] + 2, c0:c0 + (j1 - j0)]
                nc.vector.tensor_mul(
                    pt[:, :, j0:j1], pt[:, :, j0:j1], esl
                )

            def attn_pv(G, ctxps, kc, pt, stop):
                for hi, h in enumerate(G):
                    nc.tensor.matmul(
                        ctxps[hi][0:65, :],
                        v_sb[:, kc, h, 0:65],
                        pt[:, hi, :],
                        start=(kc == SEQ[0]),
                        stop=stop,
                    )

            def attn_pv_dr(G, ctxps, pr, t8, start, stop):
                ka, kb = pr
                for hi, h in enumerate(G):
                    nc.tensor.matmul(
                        ctxps[hi][0:65, :],
                        v_sb[:, ka:kb + 1:(kb - ka), h, 0:65],
                        t8[:, :, hi, :],
                        start=start,
                        stop=stop,
                        perf_mode=DR,
                    )

            def emit_pv(G, ctxps, kc):
                kind = kind_of(G, kc)
                last = kc == SEQ[-1]
                if kind == "fp8":
                    pr, j = PAIR_OF[G[0]][kc]
                    if j == 1:
                        attn_pv_dr(G, ctxps, pr, pts.pop((G[0], pr)),
                                   start=(pr[0] == SEQ[0]), stop=last)
                else:
                    attn_pv(G, ctxps, kc, pts.pop((G[0], kc)), stop=last)

            # ---- epilogue pieces ----
            def e_zrows(G, ctxps, on_scalar=True):
                """Z rows (psum partition 64) -> partition-0 f32 SBUF."""
                for hi, h in enumerate(G):
                    if on_scalar:
                        nc.scalar.copy(ztmp32[0:1, h, :], ctxps[hi][64:65, :])
                    else:
                        nc.vector.tensor_copy(
                            ztmp32[0:1, h, :], ctxps[hi][64:65, :]
                        )

            def e_ctxcopy(G, ctxps):
                gg = G[0] // 2
                for hi, h in enumerate(G):
                    po = (h % 2) * 64
                    nc.vector.tensor_copy(
                        ctx_sb[po:po + 64, gg, :], ctxps[hi][0:64, :]
                    )

            def e_recip(G):
                c = RECIP_APPROX_FAST_CONSTS
                for hi, h in enumerate(G):
                    nc.vector._custom_dve(
                        RECIPROCAL_APPROX_FAST,
                        out=rz16[0:1, h, :],
                        in0=ztmp32[0:1, h, :],
                        s0=c["s0"],
                        s1=c["s1"],
                        imm2=c["imm2"],
                    )

            def e_zscale(G, pool=None):
                """Broadcast 1/Z to each head's partitions and scale ctx."""
                gg = G[0] // 2
                if pool is None:
                    zb = mmps.tile([P, 512], f32, tag="mm", name=f"zb{gg}")
                else:
                    zb = pool.tile([P, QB], f32, tag="ctxp", name=f"zb{gg}")
                for hi, h in enumerate(G):
                    po = (h % 2) * 64
                    nc.tensor.matmul(
                        zb[po:po + 64, :],
                        ones16[0:1, :],
                        rz16[0:1, h, :],
                        start=True,
                        stop=True,
                        tile_position=(0, po),
                    )
                for hi, h in enumerate(G):
                    po = (h % 2) * 64
                    nc.vector.tensor_mul(
                        ctx_sb[po:po + 64, gg, :],
                        ctx_sb[po:po + 64, gg, :],
                        zb[po:po + 64, :],
                    )

            def e_fc0_pair(p_):
                """G0 fc for qc pair p_: psum -> fcacc (f16 SBUF)."""
                ps = mmps.tile([P, 512], f32, tag="mm", name=f"fc0p{p_}")
                for i in range(2):
                    qc = 2 * p_ + i
                    nc.tensor.matmul(
                        ps[:, i * F:(i + 1) * F],
                        ctx_sb[:, 0, qc * P:(qc + 1) * P],
                        wf_sb[:, 0, :],
                        start=True,
                        stop=True,
                    )
                nc.vector.tensor_add(
                    fcacc[:, 2 * p_:2 * p_ + 2, :].rearrange("p a b -> p (a b)"),
                    fcacc[:, 2 * p_:2 * p_ + 2, :].rearrange("p a b -> p (a b)"),
                    ps,
                )

            def e_fc1_mm(p_):
                """G1 fc + residual/fcacc via identity matmul, one qc pair.
                The ident (residual) matmul leads the group: its data is
                ready before the 1/Z scales, so PE isn't gated on them."""
                ps = mmps.tile([P, 512], f32, tag="mm", name=f"fc1p{p_}")
                for i in range(2):
                    qc = 2 * p_ + i
                    sl = ps[:, i * F:(i + 1) * F]
                    nc.tensor.matmul(
                        sl,
                        ident,
                        fcacc[:, qc, :],
                        start=True,
                        stop=False,
                        skip_group_check=True,
                    )
                    nc.tensor.matmul(
                        sl,
                        ctx_sb[:, 1, qc * P:(qc + 1) * P],
                        wf_sb[:, 1, :],
                        start=False,
                        stop=True,
                        skip_group_check=True,
                    )
                return ps

            def e_stats(p_, ps):
                for i in range(2):
                    qc = 2 * p_ + i
                    sl = ps[:, i * F:(i + 1) * F]
                    nc.vector.bn_stats(st_t[:, qc, :], sl)
                    nc.vector.bn_aggr(mv_t[:, qc, :], st_t[:, qc, :])

            def e_rstd(p_):
                q0 = 2 * p_
                sl = slice(q0, q0 + 2)
                nc.scalar.activation(
                    rstd[:, sl], mv_t[:, sl, 1], AF.Sqrt, bias=eps_t,
                    scale=1.0,
                )
                nc.vector.reciprocal(rstd[:, sl], rstd[:, sl])
                nc.vector.tensor_mul(nbias[:, sl], mv_t[:, sl, 0], rstd[:, sl])
                nc.vector.tensor_scalar_mul(nbias[:, sl], nbias[:, sl], negone)

            def e_norm(p_, ps, on_scalar):
                q0 = 2 * p_
                for i in range(2):
                    qc = 2 * p_ + i
                    if on_scalar:
                        nc.scalar.activation(
                            o_sb[:, qc, :],
                            ps[:, i * F:(i + 1) * F],
                            AF.Identity,
                            bias=nbias[:, qc:qc + 1],
                            scale=rstd[:, qc:qc + 1],
                        )
                    else:
                        nc.vector.tensor_scalar(
                            o_sb[:, qc, :],
                            ps[:, i * F:(i + 1) * F],
                            mv_t[:, qc, 0:1],
                            rstd[:, qc:qc + 1],
                            op0=ALU.subtract,
                            op1=ALU.mult,
                        )
                eng = nc.sync if p_ == 0 else nc.scalar
                eng.dma_start(
                    out[:, q0:q0 + 2, :], o_sb[:, q0:q0 + 2, :]
                )

            # ---- G0: prologue + software-pipelined loop (scores for kc+1
            # are emitted before PV of kc so dripped projections never sit
            # between a scores matmul and its EXP on the PE queue) ----
            G0, G1 = (0, 1), (2, 3)
            ctxps0 = [
                cpsum.tile([P, QB], f32, tag="ctxp", name=f"ctxp{hh}")
                for hh in G0
            ]
            proj_q(0, on_scalar=True)
            proj_k(0, 0, split_cast=True)
            emit_sc(G0, SEQ[0])
            proj_v(0)
            proj_v(1)

            post = {
                0: [lambda: proj_k(1, 0)],
                1: [lambda: proj_v(2)],
                2: [lambda: proj_v(3)],
                3: [lambda: proj_k(2, 0)],
                4: [lambda: proj_v(4)],
                5: [lambda: proj_v(5), lambda: proj_q(1)],
                6: [lambda: proj_k(3, 0)],
                7: [lambda: proj_v(6)],
                8: [lambda: proj_v(7), lambda: proj_k(0, 1)],
                9: [lambda: proj_k(1, 1)],
            }
            for i in range(KC):
                if i + 1 < KC:
                    emit_sc(G0, SEQ[i + 1])
                emit_pv(G0, ctxps0, SEQ[i])
                for step in post.get(i, []):
                    step()

            # ---- G0 -> G1 transition: 5-chunk score/exp window over the
            # drain so the psum handoff hides behind the EXP stream ----
            for i in range(5):
                emit_sc(G1, SEQ[i])
                if i == 1:
                    e_zrows(G0, ctxps0, on_scalar=True)
                elif i == 2:
                    e_ctxcopy(G0, ctxps0)
                elif i == 3:
                    e_recip(G0)
            ctxps1 = [
                cpsum.tile([P, QB], f32, tag="ctxp", name=f"ctxp{hh}")
                for hh in G1
            ]
            for i in range(4):
                emit_pv(G1, ctxps1, SEQ[i])

            # ---- G1 pipelined loop with G0-epilogue drip ----
            drip = {
                4: [lambda: proj_k(2, 1)],
                5: [lambda: e_zscale(G0)],
                6: [lambda: proj_k(3, 1)],
                7: [lambda: nc.vector.tensor_copy(fcacc, res_sb)],
                9: [lambda: e_fc0_pair(0)],
                11: [lambda: e_fc0_pair(1)],
            }
            for i in range(4, KC):
                if i + 1 < KC:
                    emit_sc(G1, SEQ[i + 1])
                emit_pv(G1, ctxps1, SEQ[i])
                for step in drip.get(i, []):
                    step()

            # ---- G1 epilogue ----
            e_zrows(G1, ctxps1)
            c_ = RECIP_APPROX_FAST_CONSTS
            nc.vector._custom_dve(
                RECIPROCAL_APPROX_FAST, out=rz16[0:1, 2, :],
                in0=ztmp32[0:1, 2, :],
                s0=c_["s0"], s1=c_["s1"], imm2=c_["imm2"],
            )
            nc.vector.tensor_copy(ctx_sb[0:64, 1, :], ctxps1[0][0:64, :])
            nc.scalar.copy(ctx_sb[64:128, 1, :], ctxps1[1][0:64, :])
            nc.vector._custom_dve(
                RECIPROCAL_APPROX_FAST, out=rz16[0:1, 3, :],
                in0=ztmp32[0:1, 3, :],
                s0=c_["s0"], s1=c_["s1"], imm2=c_["imm2"],
            )
            e_zscale(G1, pool=cpsum)
            ps0 = e_fc1_mm(0)
            ps1 = e_fc1_mm(1)
            e_stats(0, ps0)
            e_rstd(0)
            e_stats(1, ps1)
            e_norm(0, ps0, on_scalar=True)
            e_rstd(1)
            e_norm(1, ps1, on_scalar=False)

    nc.compile()
    return nc


def get_nc():
    if "nc" not in _CACHE:
        _CACHE["nc"] = _build_program()
    return _CACHE["nc"]


def make_in_maps(input_Q, input_K, input_V, W_Q, W_K, W_V, W_fc):
    import ml_dtypes

    f8 = ml_dtypes.float8_e4m3
    c16 = lambda a: np.ascontiguousarray(
        np.asarray(a, dtype=np.float32), dtype=np.float16
    )
    # pack an [in, out] matrix to SBUF layout [p, c, out]
    pk_w = lambda w: np.asarray(w, np.float32).reshape(2, P, -1).transpose(1, 0, 2)
    # pack an activation block X [seq, F] to X^T SBUF layout [p, c, seq]
    pk_t = lambda x: c16(np.asarray(x, np.float32).T.reshape(2, P, -1).transpose(1, 0, 2))
    # pack a rolled key/value matrix [2048, F] to X^T [p, nb, c, 512]
    pk_x = lambda x: np.ascontiguousarray(
        np.asarray(x, np.float32).reshape(4, 512, 2, P).transpose(3, 0, 2, 1),
        dtype=f8,
    )
    e01t, e25t = _gauss_tables()
    g01 = np.ascontiguousarray(e01t.transpose(1, 0, 2))
    g25 = np.ascontiguousarray(e25t.transpose(1, 0, 2))
    g01_neutral = np.ones_like(g01)
    w8 = np.ascontiguousarray(
        np.stack([pk_w(W_K), pk_w(W_Q), pk_w(W_V)], axis=1), dtype=f8
    )
    wf = c16(pk_w(W_fc))
    in_maps = []
    for c in range(N_CORES):
        b, qb = divmod(c, 4)
        q0 = qb * QB
        r = (q0 - 256) % S
        xq_blk = np.asarray(input_Q[b][q0:q0 + QB], np.float32)
        xk_rot = np.roll(np.asarray(input_K[b], np.float32), -r, axis=0)
        xv_rot = np.roll(np.asarray(input_V[b], np.float32), -r, axis=0)
        in_maps.append({
            "xkt": pk_x(xk_rot),
            "xqt": np.ascontiguousarray(pk_t(xq_blk), dtype=f8),
            "g01": g01_neutral if q0 == 0 else g01,
            "g25": g25,
            "res": c16(xq_blk.reshape(4, P, F).transpose(1, 0, 2)),
            "w8": w8,
            "wf": wf,
            "xvt": pk_x(xv_rot),
        })
    return in_maps


def assemble_out(results):
    out = np.empty((B, S, F), dtype=np.float32)
    for c in range(N_CORES):
        b, qb = divmod(c, 4)
        o = np.asarray(results[c]["out"], dtype=np.float32)
        out[b, qb * QB:(qb + 1) * QB, :] = o.transpose(1, 0, 2).reshape(QB, F)
    return out


def kernel(input_Q, input_K, input_V, W_Q, W_K, W_V, W_fc, attn_mask=None):
    from concourse.bass_utils import run_bass_kernel_spmd

    nc = get_nc()
    in_maps = make_in_maps(input_Q, input_K, input_V, W_Q, W_K, W_V, W_fc)
    res = run_bass_kernel_spmd(nc, in_maps, core_ids=list(range(N_CORES)))
    return assemble_out(res.results)

